# revision 27
# baseline (speedup 1.0000x reference)
"""MEGADecoder forward pass as a Bass/Tile kernel on 8 TRN2 NeuronCores.

Sharding: sequence-parallel. Each core owns SEQ/8 = 512 rows. Params are
replicated. Single-head full attention uses two bf16 AllGathers (K.T, then V)
issued as soon as each operand is produced, so the collectives overlap the
Q/V/EMA'/f/i projections and the first attention phase.

Layout: activations are stored feature-major ([8 chunks x 128 partitions,
seq 512 free]) so every GEMM is a chain of 128x128x512 PE matmuls with no
transposes anywhere:
  - projections:  out.T[o, s] = sum_d W.T[d, o] . act.T[d, s]
  - V projection: V[s, o]     = sum_d Z.T[d, s] . Wv.T[d, o]   (seq-major out)
  - scores.T:     S.T[j, i]   = sum_f K.T[f, j] . Q.T[f, i]
  - attention:    Zat.T[o, s] = sum_j V[j, o]   . P.T[j, s]
Softmax runs without max-subtraction (scores for this model are O(1)) and the
denominator comes from a ones-vector matmul accumulated across j-chunks.
All matmul operands are bf16 (PSUM accumulation stays fp32); weights are
pre-swizzled on the host into the exact SBUF layout so every weight DMA is
fully contiguous per partition. Three rotating PSUM banks are shared by the
projections, the score phase and the output head so no phase boundary waits
on a PSUM pool teardown; Zat accumulates in two 4-bank groups.
"""

import numpy as np

SEQ = 4096
D = 1024
NCORES = 8
S = SEQ // NCORES  # 512 rows per core
P = 128
FC = D // P  # 8 feature chunks
NJ = SEQ // P  # 32 j-chunks
ATT_SCALE = 1.0 / float(np.sqrt(np.float32(D)))

KT_CHUNK = P * FC * P  # one [P, FC, P] chunk of K.T, 4 per core
KT_ELEMS = D * S
V_ELEMS = S * D

_CACHE = {}


def _build_bass():
    import concourse.bacc as bacc
    import concourse.tile as tile
    import concourse.mybir as mybir

    f32 = mybir.dt.float32
    bf = mybir.dt.bfloat16
    AF = mybir.ActivationFunctionType

    nc = bacc.Bacc(None, target_bir_lowering=False, num_devices=NCORES)
    mm = nc.tensor.matmul

    # ---- DRAM I/O (all weights host-preswizzled to SBUF layout) ----
    rt = nc.dram_tensor("rt", [P, FC * (S + 1)], bf, kind="ExternalInput")
    w_in = {}
    for name, shp in [
        ("wa", [P, 8 * 2 * FC * P]), ("wd", [P, 8 * 2 * FC * P]),
        ("wz", [P, 2 * FC * 4 * P]), ("wq", [P, 2 * FC * 4 * P]),
        ("wk", [P, 2 * FC * 4 * P]), ("wv", [P, 2 * FC * 4 * P]),
        ("wema", [P, 2 * FC * 4 * P]), ("wf", [P, 2 * FC * 4 * P]),
        ("wzat", [P, 2 * FC * 4 * P]),
        ("wi", [P, FC]), ("wfin", [P, FC]),
    ]:
        w_in[name] = nc.dram_tensor(name, shp, bf, kind="ExternalInput")
    # biases packed [P, 10*FC]: rows alpha,delta,z,q(pre-scaled),k,v,ema,f,zat,i
    biases = nc.dram_tensor("biases", [P, 10 * FC], f32, kind="ExternalInput")
    bvrow = nc.dram_tensor("bvrow", [1, D], f32, kind="ExternalInput")
    out = nc.dram_tensor("out", [S, 1], f32, kind="ExternalOutput")

    with tile.TileContext(nc) as tc, \
         tc.tile_pool(name="consts", bufs=1) as consts, \
         tc.tile_pool(name="dram", bufs=1, space="DRAM") as dram, \
         tc.tile_pool(name="big", bufs=1) as big:

        row_bounce = dram.tile([2, S], f32)
        kt_in = dram.tile([KT_ELEMS], bf)
        kt_out = dram.tile([NCORES, KT_ELEMS], bf, addr_space="Shared")
        v_in = dram.tile([V_ELEMS], bf)
        v_out = dram.tile([NCORES, V_ELEMS], bf, addr_space="Shared")

        bsb = consts.tile([P, 10, FC], f32)
        ones_f32 = consts.tile([P, 1], f32)
        nc.vector.memset(ones_f32, 1.0)
        ones_bf = consts.tile([P, 1], bf)
        nc.scalar.copy(ones_bf, ones_f32)

        def bias_ap(row, chunk):
            return bsb[:, row, chunk:chunk + 1]

        # whole-kernel resident activations
        rema = big.tile([P, FC, S], bf, name="rema")
        z = big.tile([P, FC, S], bf, name="z")
        qT = big.tile([P, FC, S], bf, name="qT")
        remap = big.tile([P, FC, S], f32, name="remap")
        remapb = big.tile([P, FC, S], bf, name="remapb")
        fT = big.tile([P, FC, S], bf, name="fT")
        zatp = big.tile([P, FC, S], bf, name="zatp")
        ib = big.tile([P, S], f32, name="ib")

        # PSUM pools with hand-managed LIFO lifetimes (8 banks total).
        # pshare's 3 rotating [P,S] banks serve the projections, the score
        # phase AND the output head, so no phase boundary ever waits on a
        # pool teardown; prow's single row-bank serves ps_i, l_ps and fin_ps
        # in sequence.
        #   phase 1:      pshare(3) + prow(1) + p1ps(4) = 8
        #   projections:  pshare(3) + prow(1)           = 4  (p1ps closed)
        #   5A:           pshare(3) + prow(1)           = 4
        #   Zat A/B:      pshare(3) + prow(1) + 4       = 8
        #   head:         pshare(3) + prow(1)           = 4
        pshare_cm = tc.tile_pool(name="pshare", bufs=3, space="PSUM")
        pshare = pshare_cm.__enter__()
        prow_cm = tc.tile_pool(name="prow", bufs=1, space="PSUM")
        prow = prow_cm.__enter__()
        p6w_cm = tc.tile_pool(name="p6w", bufs=1)
        p6w = p6w_cm.__enter__()
        p2w_cm = tc.tile_pool(name="p2w", bufs=4)
        p2w = p2w_cm.__enter__()
        p1ps_cm = tc.tile_pool(name="p1ps", bufs=2, space="PSUM")
        p1ps = p1ps_cm.__enter__()

        # ---------------- Phase 1: R_EMA ----------------
        with tc.tile_pool(name="p_rt", bufs=1) as p_rt, \
             tc.tile_pool(name="p1w", bufs=4) as p1w, \
             tc.tile_pool(name="p1t", bufs=2) as p1t:
            def p1_weights(ot, split=False, eng_wd=None):
                tiles = []
                for wname, tag in (("wa", "wa"), ("wd", "wd")):
                    eng = nc.sync if (wname == "wa" or eng_wd is None) else eng_wd
                    w_t = p1w.tile([P, 2 * FC, P], bf, tag=tag)
                    base = ot * 2 * FC * P
                    halves = 2 if split else 1
                    step = 2 * FC * P // halves
                    for h in range(halves):
                        eng.dma_start(
                            out=w_t[:, h * FC:(h + 1) * FC, :] if split else w_t,
                            in_=w_in[wname].ap()[:, base + h * step:
                                                 base + (h + 1) * step]
                            .rearrange("p (c o) -> p c o", o=P))
                    tiles.append(w_t)
                return tiles

            # spread the startup DMAs across issue queues: the SP engine
            # issues serially after the entry ritual, so sync carries only
            # wa0 (the first matmul's dep); wd0 goes via scalar, rt chunks
            # alternate gpsimd/scalar
            w0 = p1_weights(0, split=True, eng_wd=nc.scalar)
            nc.scalar.dma_start(out=bsb, in_=biases.ap().rearrange(
                "p (b c) -> p b c", b=10))
            rt_sb = p_rt.tile([P, FC, S + 1], bf)
            for c in range(FC):
                nc.gpsimd.dma_start(
                    out=rt_sb[:, c, :],
                    in_=rt.ap()[:, c * (S + 1):(c + 1) * (S + 1)])
            for ot in range(FC):
                wa_t, wd_t = w0 if ot == 0 else p1_weights(ot)
                ps_a = p1ps.tile([P, S], f32, tag="psa")
                ps_d = p1ps.tile([P, S], f32, tag="psd")
                for ch in range(FC):
                    mm(ps_a, wa_t[:, ch, :], rt_sb[:, ch, 0:S],
                       start=(ch == 0), stop=False)
                    mm(ps_d, wd_t[:, ch, :], rt_sb[:, ch, 0:S],
                       start=(ch == 0), stop=False)
                for ch in range(FC):
                    mm(ps_a, wa_t[:, FC + ch, :], rt_sb[:, ch, 1:S + 1],
                       start=False, stop=(ch == FC - 1))
                    mm(ps_d, wd_t[:, FC + ch, :], rt_sb[:, ch, 1:S + 1],
                       start=False, stop=(ch == FC - 1))
                alpha_t = p1t.tile([P, S], bf, tag="alpha")
                nc.scalar.activation(alpha_t, ps_a, AF.Tanh,
                                     bias=bias_ap(0, ot), scale=1.0)
                delta_t = p1t.tile([P, S], bf, tag="delta")
                nc.scalar.activation(delta_t, ps_d, AF.Tanh,
                                     bias=bias_ap(1, ot), scale=1.0)
                # rema = t1 + alpha*(r_t - t1), t1 = delta*r_prev
                t1 = p1t.tile([P, S], bf, tag="t1")
                nc.vector.tensor_mul(t1, delta_t, rt_sb[:, ot, 0:S])
                t2 = p1t.tile([P, S], bf, tag="t2")
                nc.vector.tensor_sub(t2, rt_sb[:, ot, 1:S + 1], t1)
                t3 = p1t.tile([P, S], bf, tag="t3")
                nc.vector.tensor_mul(t3, alpha_t, t2)
                nc.vector.tensor_add(rema[:, ot, :], t3, t1)

        # ------- Phase 2+3: Z, K.T (+AG), Q.T, V (+AG), EMA', f, i -------
        with tc.tile_pool(name="p_kv", bufs=1) as p_kv, \
             tc.tile_pool(name="p_vsb", bufs=4) as p_vsb, \
             tc.tile_pool(name="p4t", bufs=1) as p4t:
            def proj(w_name, rhs_src, out_tile, func, bias_row, scale=1.0,
                     out_tile2=None):
                for half in range(2):
                    w_t = p2w.tile([P, FC, 4 * P], bf, tag="w")
                    nc.sync.dma_start(
                        out=w_t,
                        in_=w_in[w_name].ap()[:, half * FC * 4 * P:
                                              (half + 1) * FC * 4 * P]
                        .rearrange("p (c o) -> p c o", c=FC))
                    for sub in range(4):
                        ot = half * 4 + sub
                        ow = slice(sub * P, (sub + 1) * P)
                        ps = pshare.tile([P, S], f32, tag="ps")
                        for ch in range(FC):
                            mm(ps, w_t[:, ch, ow], rhs_src[:, ch, :],
                               start=(ch == 0), stop=(ch == FC - 1))
                        nc.scalar.activation(out_tile[:, ot, :], ps, func,
                                             bias=bias_ap(bias_row, ot),
                                             scale=scale)
                        if out_tile2 is not None:
                            nc.vector.tensor_copy(out_tile2[:, ot, :],
                                                  out_tile[:, ot, :])

            proj("wz", rema, z, AF.Silu, 2)

            # K.T first, straight into the collective input (per-ot DMA)
            ktS = p_kv.tile([P, FC, S], bf)
            proj("wk", z, ktS, AF.Identity, 4)
            kt_in_ap = kt_in[:].rearrange("(cl p c s) -> p c cl s",
                                          cl=4, p=P, c=FC)
            for ot in range(FC):
                nc.gpsimd.dma_start(
                    out=kt_in_ap[:, ot, :, :],
                    in_=ktS[:, ot, :].rearrange("p (cl s) -> p cl s", cl=4))
            nc.gpsimd.collective_compute(
                "AllGather", mybir.AluOpType.bypass,
                replica_groups=[list(range(NCORES))],
                ins=[kt_in[:].opt()], outs=[kt_out[:].opt()],
            )

            # Q.T fills the PE while the K.T AllGather is in flight
            proj("wq", z, qT, AF.Identity, 3, scale=ATT_SCALE)

            # V seq-major: V[s, o] = sum_d Z.T[d, s] Wv.T[d, o] (+ bv)
            bv_b = p_kv.tile([P, D], f32, tag="bvb")
            nc.scalar.dma_start(out=bv_b, in_=bvrow.ap().partition_broadcast(P))
            for half in range(2):
                osl = slice(half * 4 * P, (half + 1) * 4 * P)
                wv_t = p2w.tile([P, FC, 4 * P], bf, tag="w")
                nc.sync.dma_start(
                    out=wv_t,
                    in_=w_in["wv"].ap()[:, half * FC * 4 * P:
                                        (half + 1) * FC * 4 * P]
                    .rearrange("p (c o) -> p c o", c=FC))
                for st in range(4):
                    ssl = slice(st * P, (st + 1) * P)
                    ps = p1ps.tile([P, 4 * P], f32, tag="psa")
                    for ch in range(FC):
                        mm(ps, z[:, ch, ssl], wv_t[:, ch, :],
                           start=(ch == 0), stop=(ch == FC - 1))
                    v_sb = p_vsb.tile([P, 4 * P], bf, tag="vsb")
                    nc.vector.tensor_add(v_sb, ps, bv_b[:, osl])
                    nc.gpsimd.dma_start(
                        out=v_in[st * P * D:(st + 1) * P * D].rearrange(
                            "(p o) -> p o", p=P)[:, osl],
                        in_=v_sb)
            nc.gpsimd.collective_compute(
                "AllGather", mybir.AluOpType.bypass,
                replica_groups=[list(range(NCORES))],
                ins=[v_in[:].opt()], outs=[v_out[:].opt()],
            )

            # EMA', f, i projections run while the AllGathers drain.
            proj("wema", rema, remap, AF.Identity, 6, out_tile2=remapb)
            proj("wf", remapb, fT, AF.Sigmoid, 7)

            wi_sb = p4t.tile([P, FC], bf, tag="wi")
            nc.sync.dma_start(out=wi_sb, in_=w_in["wi"].ap())
            ps_i = prow.tile([1, S], f32, tag="rowps")
            for ch in range(FC):
                mm(ps_i, wi_sb[:, ch:ch + 1], rema[:, ch, :],
                   start=(ch == 0), stop=(ch == FC - 1))
            i_row = p4t.tile([1, S], f32, tag="irow")
            nc.scalar.activation(i_row, ps_i, AF.Tanh,
                                 bias=bsb[0:1, 9, 0:1], scale=1.0)
            nc.gpsimd.dma_start(out=row_bounce[0:1, :], in_=i_row)
            nc.gpsimd.dma_start(
                out=ib, in_=row_bounce[0:1, :].partition_broadcast(P))
        p1ps_cm.__exit__(None, None, None)
        p2w_cm.__exit__(None, None, None)

        # ------- Phases 5+6: attention + output head -------
        # phase-6 weights prefetch early (no deps, hides under collectives)
        w6 = []
        for half in range(2):
            w_t = p6w.tile([P, FC, 4 * P], bf, tag=f"wzat{half}")
            nc.sync.dma_start(
                out=w_t,
                in_=w_in["wzat"].ap()[:, half * FC * 4 * P:
                                      (half + 1) * FC * 4 * P]
                .rearrange("p (c o) -> p c o", c=FC))
            w6.append(w_t)
        wfin_sb = p6w.tile([P, FC], bf, tag="wfin")
        nc.sync.dma_start(out=wfin_sb, in_=w_in["wfin"].ap())

        with tc.tile_pool(name="p_pt", bufs=1) as p_pt, \
             tc.tile_pool(name="p_rl", bufs=1) as p_rl, \
             tc.tile_pool(name="p5v", bufs=8) as p5v:
            pt = p_pt.tile([P, NJ, S], bf)
            rl_b = p_rl.tile([P, S], f32, tag="rlb")

            def vch_dma(jc, tag):
                r, cl = jc // 4, jc % 4
                v_ch = p5v.tile([P, D], bf, tag=tag)
                nc.gpsimd.dma_start(
                    out=v_ch,
                    in_=v_out[r, cl * P * D:(cl + 1) * P * D]
                    .rearrange("(p o) -> p o", p=P))
                return v_ch

            # --- 5A: scores.T + exp + denominator ---
            with tc.tile_pool(name="p5w", bufs=8) as p5w:
                l_ps = prow.tile([1, S], f32, tag="rowps")
                for jc in range(NJ):
                    r, cl = jc // 4, jc % 4
                    kt_ch = p5w.tile([P, FC, P], bf, tag="ktch")
                    nc.sync.dma_start(
                        out=kt_ch,
                        in_=kt_out[r, cl * KT_CHUNK:(cl + 1) * KT_CHUNK]
                        .rearrange("(p c s) -> p c s", p=P, c=FC))
                    s_ps = pshare.tile([P, S], f32, tag="ps")
                    for ch in range(FC):
                        mm(s_ps, kt_ch[:, ch, :], qT[:, ch, :],
                           start=(ch == 0), stop=(ch == FC - 1))
                    nc.scalar.activation(pt[:, jc, :], s_ps, AF.Exp,
                                         bias=0.0, scale=1.0)
                    if jc >= 1:
                        mm(l_ps, ones_bf, pt[:, jc - 1, :],
                           start=(jc - 1 == 0), stop=False)
                mm(l_ps, ones_bf, pt[:, NJ - 1, :],
                   start=False, stop=True)
                l_row = p_rl.tile([1, S], f32, tag="lrow")
                nc.vector.reciprocal(l_row, l_ps)
                nc.scalar.dma_start(out=row_bounce[1:2, :], in_=l_row)
                nc.scalar.dma_start(
                    out=rl_b,
                    in_=row_bounce[1:2, :].partition_broadcast(P))

            # --- 5B: Zat.T in two 4-bank PSUM groups ---
            zat_ps = [None] * FC
            pvpsA_cm = tc.tile_pool(name="pvpsA", bufs=1, space="PSUM")
            pvpsA = pvpsA_cm.__enter__()
            for i in range(4):
                zat_ps[i] = pvpsA.tile([P, S], f32, tag=f"zat{i}",
                                       name=f"zat{i}")
            for ja in range(NJ):
                v_ch = vch_dma(ja, "vchA")
                for ot in range(4):
                    mm(zat_ps[ot], v_ch[:, ot * P:(ot + 1) * P],
                       pt[:, ja, :],
                       start=(ja == 0), stop=(ja == NJ - 1))
            # group A epilogue: zatp = f * (zat/l)
            for ot in range(4):
                nc.vector.tensor_mul(zat_ps[ot], zat_ps[ot], rl_b)
                nc.vector.tensor_mul(zatp[:, ot, :], zat_ps[ot],
                                     fT[:, ot, :])
            pvpsA_cm.__exit__(None, None, None)

            with tc.tile_pool(name="pvpsB", bufs=1, space="PSUM") as pvpsB:
                for i in range(4, FC):
                    zat_ps[i] = pvpsB.tile([P, S], f32, tag=f"zat{i}",
                                           name=f"zat{i}")
                for jb in range(NJ):
                    v_ch = vch_dma(jb, "vchB")
                    for ot in range(4, FC):
                        mm(zat_ps[ot], v_ch[:, ot * P:(ot + 1) * P],
                           pt[:, jb, :],
                           start=(jb == 0), stop=(jb == NJ - 1))
                for ot in range(4, FC):
                    nc.vector.tensor_mul(zat_ps[ot], zat_ps[ot], rl_b)
                    nc.vector.tensor_mul(zatp[:, ot, :], zat_ps[ot],
                                         fT[:, ot, :])

        # ---------------- Phase 6: output head ----------------
        # fin matmuls are hoisted after the ot loop so the in-order PE
        # stream never waits mid-phase on a zf DVE chain; all 8 zf tiles
        # stay live in their own pool.
        with tc.tile_pool(name="p6t", bufs=3) as p6t, \
             tc.tile_pool(name="p6zf", bufs=8) as p6zf:
            fin_ps = prow.tile([1, S], f32, tag="rowps")
            zfs = []
            for half in range(2):
                for sub in range(4):
                    ot = half * 4 + sub
                    ow = slice(sub * P, (sub + 1) * P)
                    ps = pshare.tile([P, S], f32, tag="ps")
                    for ch in range(FC):
                        mm(ps, w6[half][:, ch, ow], zatp[:, ch, :],
                           start=(ch == 0), stop=(ch == FC - 1))
                    t_sum = p6t.tile([P, S], f32, tag="tsum")
                    nc.vector.tensor_add(t_sum, ps, remap[:, ot, :])
                    ztp = p6t.tile([P, S], f32, tag="ztp")
                    nc.scalar.activation(ztp, t_sum, AF.Tanh,
                                         bias=bias_ap(8, ot), scale=1.0)
                    # zf = remap + ib*(ztp - remap)
                    d_t = p6t.tile([P, S], f32, tag="dt")
                    nc.vector.tensor_sub(d_t, ztp, remap[:, ot, :])
                    m_t = p6t.tile([P, S], f32, tag="mt")
                    nc.vector.tensor_mul(m_t, d_t, ib)
                    zf = p6zf.tile([P, S], bf, tag="zf")
                    nc.vector.tensor_add(zf, m_t, remap[:, ot, :])
                    zfs.append(zf)
            for ot in range(FC):
                mm(fin_ps, wfin_sb[:, ot:ot + 1], zfs[ot],
                   start=(ot == 0), stop=(ot == FC - 1))
            phat = p6t.tile([1, S], f32, tag="phat")
            nc.scalar.activation(phat, fin_ps, AF.Sigmoid,
                                 bias=0.0, scale=1.0)
            nc.sync.dma_start(out=out.ap().rearrange("s o -> o s"),
                              in_=phat)

        p6w_cm.__exit__(None, None, None)
        prow_cm.__exit__(None, None, None)
        pshare_cm.__exit__(None, None, None)
    nc.finalize()
    return nc


def _prep_host_inputs(inputs):
    """Swizzle weights into the exact SBUF layouts (pure host layout work)."""
    import ml_dtypes
    bf = ml_dtypes.bfloat16

    R = np.ascontiguousarray(inputs["R"], dtype=np.float32)
    RT_ext = np.concatenate(
        [np.zeros((D, 1), np.float32), np.ascontiguousarray(R.T)], axis=1)
    # [D, S+1] per core -> [P, FC*(S+1)]
    rt_cores = []
    for c in range(NCORES):
        blk = RT_ext[:, c * S:c * S + S + 1]  # [D, S+1]
        rt_cores.append(np.ascontiguousarray(
            blk.reshape(FC, P, S + 1).transpose(1, 0, 2)
            .reshape(P, FC * (S + 1))).astype(bf))

    def sw_big(wT):  # [2D, D] -> [P, 8*2FC*P]  (per-ot [2FC, P] blocks)
        return np.ascontiguousarray(
            wT.reshape(2 * FC, P, FC, P).transpose(1, 2, 0, 3)
            .reshape(P, FC * 2 * FC * P)).astype(bf)

    def sw_sq(wT):  # [D, D] -> [P, 2*FC*4P]  (per-half [FC, 512] blocks)
        return np.ascontiguousarray(
            wT.reshape(FC, P, 2, 4 * P).transpose(1, 2, 0, 3)
            .reshape(P, 2 * FC * 4 * P)).astype(bf)

    def sw_col(wT):  # [D, 1] -> [P, FC]
        return np.ascontiguousarray(
            wT.reshape(FC, P).T).astype(bf)

    w = {
        "wa": sw_big(inputs["W_alpha"].T.astype(np.float32)),
        "wd": sw_big(inputs["W_delta"].T.astype(np.float32)),
        "wz": sw_sq(inputs["W_z"].T.astype(np.float32)),
        "wq": sw_sq(inputs["W_q"].T.astype(np.float32)),
        "wk": sw_sq(inputs["W_k"].T.astype(np.float32)),
        "wv": sw_sq(inputs["W_v"].T.astype(np.float32)),
        "wema": sw_sq(inputs["W_EMA"].T.astype(np.float32)),
        "wf": sw_sq(inputs["W_f"].T.astype(np.float32)),
        "wzat": sw_sq(inputs["W_z_at"].T.astype(np.float32)),
        "wi": sw_col(inputs["W_i"].T.astype(np.float32)),
        "wfin": sw_col(inputs["W_final"].T.astype(np.float32)),
    }

    braw = np.zeros((10, D), np.float32)
    braw[0] = inputs["b_alpha"]
    braw[1] = inputs["b_delta"]
    braw[2] = inputs["b_z"]
    braw[3] = inputs["b_q"] * ATT_SCALE
    braw[4] = inputs["b_k"]
    braw[5] = inputs["b_v"]
    braw[6] = inputs["b_EMA"]
    braw[7] = inputs["b_f"]
    braw[8] = inputs["b_z_at"]
    braw[9, 0] = np.float32(inputs["b_i"][0])
    # [10, D] -> [P, 10*FC]  (bsb[p, row, c] = braw[row, c*P+p])
    biases = np.ascontiguousarray(
        braw.reshape(10, FC, P).transpose(2, 0, 1).reshape(P, 10 * FC))
    bvrow = np.ascontiguousarray(braw[5:6, :])

    in_maps = []
    for c in range(NCORES):
        m = {"rt": rt_cores[c], "biases": biases, "bvrow": bvrow}
        m.update(w)
        in_maps.append(m)
    return in_maps


def kernel(**inputs):
    from concourse.bass_utils import run_bass_kernel_spmd

    if "nc" not in _CACHE:
        _CACHE["nc"] = _build_bass()
    nc = _CACHE["nc"]
    in_maps = _prep_host_inputs(inputs)
    res = run_bass_kernel_spmd(nc, in_maps, core_ids=list(range(NCORES)))
    outs = [res.results[c]["out"] for c in range(NCORES)]
    return np.concatenate(outs, axis=0).astype(np.float32)
